# revision 9
# baseline (speedup 1.0000x reference)
"""Trainium2 Bass kernel for nn_Base2DInference (sampling).

Data-parallel over the sample batch B across 8 NeuronCores. Per core:
  - tiny MLP 10->32->32->32->32 fp32 on the PE (4 sample-groups packed into
    the 128x128 array), PE transpose to samples-on-partitions layout
  - rotation + texel index math on DVE/ACT; z bin via logit thresholds on the
    raw MLP output (no sigmoid, no ACT table thrash)
  - fac / fac*norm per-bin values via is_ge staircase selects (no gather);
    texture norms computed f32 on DVE from the granule table stream
  - texel fetch: per-d granule tables pdf_gr[d] = [32768, 64] f32 (256B
    granules); int16 granule indices brought into the SWDGE wrapped layout by
    contiguous partition-fold DMAs + replicate + one DVE free-axis reorder,
    then dma_gather (4096 idx/call) cycled over 4 SWDGE queues
  - one-hot extraction of the texel from each 64-wide granule on DVE
  - quarter-passes: each quarter's gathers overlap the next quarter's MLP
"""
import sys, os, types

sys.path.insert(0, '/opt/trn_rl_repo')

import numpy as np


def _install_ntff_hook_shim():
    if 'antenv.axon_hooks' in sys.modules:
        return
    try:
        from trn_agent_boot.trn_boot import _ntff_profile_via_ctypes
        hook = _ntff_profile_via_ctypes('/opt/axon/libaxon_pjrt.so')
    except Exception:
        hook = None
    mod = types.ModuleType('antenv.axon_hooks')
    _state = {'hook': hook}
    mod.set_axon_ntff_profile_hook = lambda h: _state.__setitem__('hook', h)
    mod.get_axon_ntff_profile_hook = lambda: _state['hook']
    sys.modules['antenv.axon_hooks'] = mod


_install_ntff_hook_shim()

import math as _math

from concourse import bass, mybir, bacc, tile
from concourse.bass_utils import run_bass_kernel_spmd

F32 = mybir.dt.float32
BF16 = mybir.dt.bfloat16
I32 = mybir.dt.int32
I16 = mybir.dt.int16

RES, ANG, D, HID, CIN = 512, 8, 8, 32, 10
B = 1048576
NC_N = 8
BC = B // NC_N            # samples per core: 131072
NQ = 4                    # packed sample groups per matmul
QS = BC // NQ             # samples per group: 32768
NT = QS // 512            # super-tiles: 64
NCHUNK = BC // 128        # 128-sample chunks: 1024
GRAN = 64                 # texels per granule (256B f32)
NGR_D = ANG * RES * RES // GRAN   # granules per d-table: 32768
CALL_IDX = 4096           # indices per dma_gather call
CALL_CH = CALL_IDX // 128  # CH columns per call: 32
NQT = 8                   # pass count
QT_CH = NCHUNK // NQT     # CH per quarter: 256
QT_ST = NT // NQT         # supertiles per quarter: 16
NCALL_QT = QT_CH // CALL_CH   # calls per (quarter, d): 8

# z-bin thresholds: z_idx >= k  <=>  zz >= logit((k-0.5)/8)
LOGIT = [0.0] + [float(_math.log((k - 0.5) / (8.0 - (k - 0.5))))
                 for k in range(1, ANG)]


def build_kernel():
    AL = mybir.AluOpType
    AF = mybir.ActivationFunctionType
    nc = bacc.Bacc(dynamic_dma_scratch_size=36864, num_swdge_queues=4)

    cond_t = nc.declare_dram_parameter("cond_t", [NQ * CIN, QS], F32, isOutput=False)
    wi_xy = nc.declare_dram_parameter("wi_xy", [128, NT * 256], F32, isOutput=False)
    w0p = nc.declare_dram_parameter("w0p", [128, 128], F32, isOutput=False)
    w1p = nc.declare_dram_parameter("w1p", [128, 128], F32, isOutput=False)
    w2p = nc.declare_dram_parameter("w2p", [128, 128], F32, isOutput=False)
    w3p = nc.declare_dram_parameter("w3p", [128, 128], F32, isOutput=False)
    b012 = nc.declare_dram_parameter("b012", [128, 3], F32, isOutput=False)
    b3p = nc.declare_dram_parameter("b3p", [128, 1], F32, isOutput=False)
    ident = nc.declare_dram_parameter("ident", [128, 128], F32, isOutput=False)
    iota64 = nc.declare_dram_parameter("iota64", [128, GRAN], F32, isOutput=False)
    fac_in = nc.declare_dram_parameter("fac_in", [1, ANG * D], F32, isOutput=False)
    pdf_gr = nc.declare_dram_parameter("pdf_gr", [D * NGR_D, GRAN], F32,
                                       isOutput=False)
    out_ext = nc.declare_dram_parameter("out", [128, NCHUNK], F32, isOutput=True)

    with tile.TileContext(nc) as tc:
        with (
            tc.tile_pool(name="const", bufs=1) as cpool,
            tc.tile_pool(name="work", bufs=2) as wpool,
            tc.tile_pool(name="math", bufs=2) as mpool,
            tc.tile_pool(name="hrep", bufs=1) as hpool,
            tc.tile_pool(name="idx", bufs=2) as ipool,
            tc.tile_pool(name="gath", bufs=6) as gpool,
            tc.tile_pool(name="extr", bufs=2) as epool,
            tc.tile_pool(name="psum", bufs=2, space="PSUM") as ppool,
            tc.tile_pool(name="psum2", bufs=2, space="PSUM") as ppool2,
        ):
            # ---- constants to SBUF ----
            w0t = cpool.tile([128, 128], F32); nc.sync.dma_start(w0t[:], w0p[:])
            w1t = cpool.tile([128, 128], F32); nc.sync.dma_start(w1t[:], w1p[:])
            w2t = cpool.tile([128, 128], F32); nc.sync.dma_start(w2t[:], w2p[:])
            w3t = cpool.tile([128, 128], F32); nc.sync.dma_start(w3t[:], w3p[:])
            bt = cpool.tile([128, 3], F32); nc.sync.dma_start(bt[:], b012[:])
            b3t = cpool.tile([128, 1], F32); nc.sync.dma_start(b3t[:], b3p[:])
            idt = cpool.tile([128, 128], F32); nc.sync.dma_start(idt[:], ident[:])
            iof = cpool.tile([128, GRAN], F32); nc.sync.dma_start(iof[:], iota64[:])
            iob = cpool.tile([128, GRAN], BF16)
            nc.vector.tensor_copy(iob[:], iof[:])
            facr = cpool.tile([1, ANG * D], F32)
            nc.sync.dma_start(facr[:], fac_in[:])
            rhs = cpool.tile([128, 512], F32)
            nc.vector.memset(rhs[:], 0.0)
            eps24 = cpool.tile([128, 1], F32)
            nc.vector.memset(eps24[:], 1e-24)
            c256 = cpool.tile([128, 1], F32)
            nc.vector.memset(c256[:], 256.0)
            onesf = cpool.tile([128, 1], F32)
            nc.vector.memset(onesf[:], 1.0)

            # ---- per-texture sums in f32 from the granule table ----
            # texture t=(z,d): granule rows d*32768+z*4096 .. +4096 = 1MB
            sacc = cpool.tile([128, ANG * D], F32)
            for d in range(D):
                for z in range(ANG):
                    st_ = wpool.tile([128, 2048], F32, tag="nsum")
                    base = d * NGR_D + z * 4096
                    nc.sync.dma_start(
                        st_[:],
                        pdf_gr[base:base + 4096, :].rearrange(
                            "(p a) g -> p (a g)", p=128))
                    t = z * D + d
                    # two-level reduce limits sequential-add rounding on
                    # catastrophically-cancelling texture sums
                    p1 = wpool.tile([128, 32, 1], F32, tag="nsum1")
                    nc.vector.tensor_reduce(
                        p1[:], st_[:].rearrange("p (a g) -> p a g", g=64),
                        axis=mybir.AxisListType.X, op=AL.add)
                    nc.vector.tensor_reduce(sacc[:, t:t + 1],
                                            p1[:].rearrange("p a o -> p (a o)"),
                                            axis=mybir.AxisListType.X, op=AL.add)
            psum_s = ppool2.tile([1, ANG * D], F32, space="PSUM", tag="pssum")
            nc.tensor.matmul(psum_s[:], onesf[:], sacc[:], start=True, stop=True)
            srow = cpool.tile([1, ANG * D], F32)
            nc.vector.tensor_copy(srow[:], psum_s[:])
            # norm = (RES*RES/4) / max(sum, 1e-12)
            nsr = cpool.tile([1, ANG * D], F32)
            nc.vector.tensor_scalar_max(nsr[:], srow[:], 1e-12)
            nc.vector.reciprocal(nsr[:], nsr[:])
            nc.vector.tensor_scalar_mul(nsr[:], nsr[:], float(RES * RES) / 4.0)
            # fn = fac * norm row
            fnr = cpool.tile([1, ANG * D], F32)
            nc.vector.tensor_tensor(fnr[:], facr[:], nsr[:], op=AL.mult)
            fact = cpool.tile([128, ANG * D], F32)
            nc.gpsimd.partition_broadcast(fact[:], facr[:], channels=128)
            fnt = cpool.tile([128, ANG * D], F32)
            nc.gpsimd.partition_broadcast(fnt[:], fnr[:], channels=128)
            # staircase diffs along z (col = z*D + d)
            fdd = cpool.tile([128, ANG * D], F32)
            nc.vector.tensor_copy(fdd[:, 0:D], fact[:, 0:D])
            fnd = cpool.tile([128, ANG * D], F32)
            nc.vector.tensor_copy(fnd[:, 0:D], fnt[:, 0:D])
            for k in range(1, ANG):
                nc.vector.tensor_tensor(fdd[:, k * D:(k + 1) * D],
                                        fact[:, k * D:(k + 1) * D],
                                        fact[:, (k - 1) * D:k * D], op=AL.subtract)
                nc.vector.tensor_tensor(fnd[:, k * D:(k + 1) * D],
                                        fnt[:, k * D:(k + 1) * D],
                                        fnt[:, (k - 1) * D:k * D], op=AL.subtract)

            # ---- big state buffers ----
            ot = cpool.tile([128, NCHUNK], F32)
            nc.vector.memset(ot[:], 0.0)
            q_all = cpool.tile([128, NCHUNK * D], BF16)   # (CH, f) interleaved
            r_all = cpool.tile([128, NCHUNK * D], BF16)   # (CH, f) interleaved
            g16_all = cpool.tile([128, NCHUNK * D], I16)  # d-major: d*1024 + CH

            iob3 = iob[:].rearrange("p (o g) -> p o g", o=1)
            kctr = 0

            for qt in range(NQT):
                # ===== quarter phase 1: MLP + index math =====
                for si in range(QT_ST):
                    s = qt * QT_ST + si
                    nc.sync.dma_start(rhs[0:NQ * CIN, :],
                                      cond_t[:, s * 512:(s + 1) * 512])
                    h = rhs
                    for li, wt_ in enumerate((w0t, w1t, w2t, w3t)):
                        mm = ppool.tile([128, 512], F32, space="PSUM", tag="mm")
                        nc.tensor.matmul(mm[:], wt_[:], h[:], start=True, stop=True)
                        hn = wpool.tile([128, 512], F32, tag=f"h{li % 2}")
                        if li < 3:
                            nc.scalar.activation(hn[:], mm[:], AF.Relu,
                                                 bias=bt[:, li:li + 1], scale=1.0)
                        else:
                            nc.scalar.activation(hn[:], mm[:], AF.Identity,
                                                 bias=b3t[:, 0:1], scale=1.0)
                        h = hn

                    tp = ppool.tile([128, 512], F32, space="PSUM", tag="tp")
                    for c in range(4):
                        nc.tensor.transpose(
                            tp[:, c * 128:(c + 1) * 128],
                            h[:, c * 128:(c + 1) * 128],
                            idt[:])
                    tps = wpool.tile([128, 512], F32, tag="tps")
                    nc.scalar.activation(tps[:], tp[:], AF.Copy)

                    def blk(base):
                        return tps[:].rearrange(
                            "p (ch f) -> p ch f", ch=16)[:, :, base:base + 8]

                    WT, VX, VY, ZZ = blk(0), blk(8), blk(16), blk(24)
                    cw = s * 128
                    wl = wpool.tile([128, 256], F32, tag="wl")
                    nc.sync.dma_start(wl[:], wi_xy[:, s * 256:(s + 1) * 256])
                    wxs3 = wl[:, 0:128].rearrange("p (ch f) -> p ch f", ch=16)
                    wys3 = wl[:, 128:256].rearrange("p (ch f) -> p ch f", ch=16)

                    t1 = mpool.tile([128, 16, 8], F32, tag="t1")
                    t2 = mpool.tile([128, 16, 8], F32, tag="t2")
                    n2 = mpool.tile([128, 16, 8], F32, tag="n2")
                    inv = mpool.tile([128, 16, 8], F32, tag="inv")
                    nc.scalar.activation(t1[:], VX, AF.Square)
                    nc.scalar.activation(t2[:], VY, AF.Square)
                    nc.vector.tensor_tensor(n2[:], t1[:], t2[:], op=AL.add)
                    nc.scalar.activation(n2[:], n2[:], AF.Sqrt, bias=eps24[:, 0:1])
                    nc.vector.reciprocal(inv[:], n2[:])

                    rx = mpool.tile([128, 16, 8], F32, tag="rx")
                    ry = mpool.tile([128, 16, 8], F32, tag="ry")
                    nc.vector.tensor_tensor(t1[:], VX, wxs3, op=AL.mult)
                    nc.vector.tensor_tensor(t2[:], VY, wys3, op=AL.mult)
                    nc.vector.tensor_tensor(rx[:], t1[:], t2[:], op=AL.subtract)
                    nc.vector.tensor_tensor(t1[:], VY, wxs3, op=AL.mult)
                    nc.vector.tensor_tensor(t2[:], VX, wys3, op=AL.mult)
                    nc.vector.tensor_tensor(ry[:], t1[:], t2[:], op=AL.add)
                    nc.vector.tensor_tensor(rx[:], rx[:], inv[:], op=AL.mult)
                    nc.vector.tensor_tensor(ry[:], ry[:], inv[:], op=AL.mult)
                    nc.scalar.activation(t1[:], rx[:], AF.Relu,
                                         bias=c256[:, 0:1], scale=256.0)
                    nc.scalar.activation(t2[:], ry[:], AF.Relu,
                                         bias=c256[:, 0:1], scale=256.0)
                    nc.vector.tensor_scalar_min(t1[:], t1[:], 511.0)
                    nc.vector.tensor_scalar_min(t2[:], t2[:], 511.0)

                    xf = mpool.tile([128, 16, 8], F32, tag="xf")
                    yf = mpool.tile([128, 16, 8], F32, tag="yf")
                    xh = mpool.tile([128, 16, 8], F32, tag="xh")
                    ti = mpool.tile([128, 16, 8], I32, tag="ti")
                    fmk = mpool.tile([128, 16, 8], F32, tag="fmk")

                    def exact_floor(dst_f, src_f):
                        nc.vector.tensor_copy(ti[:], src_f)
                        nc.vector.tensor_copy(dst_f[:], ti[:])
                        nc.vector.tensor_tensor(fmk[:], dst_f[:], src_f, op=AL.is_gt)
                        nc.vector.tensor_tensor(dst_f[:], dst_f[:], fmk[:],
                                                op=AL.subtract)

                    exact_floor(xf, t1[:])
                    exact_floor(yf, t2[:])

                    # z bin zf = sum of is_ge(ZZ, logit_k); staircases off ZZ
                    zf = mpool.tile([128, 16, 8], F32, tag="zf")
                    nc.vector.tensor_scalar(zf[:], ZZ, LOGIT[1], 0.0,
                                            op0=AL.is_ge, op1=AL.add)
                    fcv = mpool.tile([128, 16, 8], F32, tag="fcv")
                    fnv = mpool.tile([128, 16, 8], F32, tag="fnv")
                    fd0 = fdd[:, 0:D].rearrange(
                        "p (o f) -> p o f", o=1).to_broadcast([128, 16, 8])
                    fn0 = fnd[:, 0:D].rearrange(
                        "p (o f) -> p o f", o=1).to_broadcast([128, 16, 8])
                    nc.vector.tensor_copy(fcv[:], fd0)
                    nc.vector.tensor_copy(fnv[:], fn0)
                    tmpm = mpool.tile([128, 16, 8], F32, tag="tmpm")
                    for k in range(1, ANG):
                        fdk = fdd[:, k * D:(k + 1) * D].rearrange(
                            "p (o f) -> p o f", o=1).to_broadcast([128, 16, 8])
                        fnk = fnd[:, k * D:(k + 1) * D].rearrange(
                            "p (o f) -> p o f", o=1).to_broadcast([128, 16, 8])
                        nc.vector.scalar_tensor_tensor(tmpm[:], ZZ, LOGIT[k], fdk,
                                                       op0=AL.is_ge, op1=AL.mult)
                        nc.vector.tensor_tensor(fcv[:], fcv[:], tmpm[:], op=AL.add)
                        nc.vector.scalar_tensor_tensor(tmpm[:], ZZ, LOGIT[k], fnk,
                                                       op0=AL.is_ge, op1=AL.mult)
                        nc.vector.tensor_tensor(fnv[:], fnv[:], tmpm[:], op=AL.add)
                        if k >= 2:
                            nc.vector.scalar_tensor_tensor(zf[:], ZZ, LOGIT[k],
                                                           zf[:], op0=AL.is_ge,
                                                           op1=AL.add)

                    # granule g = zf*4096 + yf*8 + floor(xf/64); r = xf - 64*
                    nc.vector.tensor_scalar_mul(t1[:], xf[:], 1.0 / 64.0)
                    exact_floor(xh, t1[:])
                    r3 = r_all[:, cw:cw + 128].rearrange(
                        "p (ch f) -> p ch f", ch=16)
                    nc.vector.scalar_tensor_tensor(r3, xh[:], -64.0, xf[:],
                                                   op0=AL.mult, op1=AL.add)
                    g1 = mpool.tile([128, 16, 8], F32, tag="g1")
                    nc.vector.scalar_tensor_tensor(g1[:], yf[:], 8.0, xh[:],
                                                   op0=AL.mult, op1=AL.add)
                    nc.vector.scalar_tensor_tensor(g1[:], zf[:], 4096.0, g1[:],
                                                   op0=AL.mult, op1=AL.add)
                    # d-major int16 writes
                    for d in range(D):
                        gsl = g16_all[:, d * NCHUNK + s * 16:
                                      d * NCHUNK + (s + 1) * 16].rearrange(
                            "p (ch o) -> p ch o", o=1)
                        nc.vector.tensor_copy(gsl, g1[:, :, d:d + 1])

                    # rl, |fac|, den, q
                    rl = mpool.tile([128, 16, 8], F32, tag="rl")
                    nc.vector.tensor_scalar_max(rl[:], WT, 0.0)
                    afc = mpool.tile([128, 16, 8], F32, tag="afc")
                    nc.vector.tensor_scalar_mul(afc[:], fcv[:], -1.0)
                    nc.vector.tensor_tensor(afc[:], afc[:], fcv[:], op=AL.max)
                    aw = mpool.tile([128, 16, 8], F32, tag="aw")
                    nc.vector.tensor_tensor(aw[:], rl[:], afc[:], op=AL.mult)
                    den = mpool.tile([128, 16, 1], F32, tag="den")
                    nc.vector.tensor_reduce(den[:], aw[:],
                                            axis=mybir.AxisListType.X, op=AL.add)
                    nc.vector.tensor_scalar_max(den[:], den[:], 1e-12)
                    nc.vector.reciprocal(den[:], den[:])
                    qv = mpool.tile([128, 16, 8], F32, tag="qv")
                    nc.vector.tensor_tensor(qv[:], rl[:], fnv[:], op=AL.mult)
                    q3 = q_all[:, cw:cw + 128].rearrange(
                        "p (ch f) -> p ch f", ch=16)
                    nc.vector.tensor_tensor(q3, qv[:],
                                            den[:].to_broadcast([128, 16, 8]),
                                            op=AL.mult)

                # ===== quarter phase 2: fold + gather + extract =====
                for d in range(D):
                    # partition-fold with contiguous runs:
                    # hrep[r, q*256+ch] = g16[q*16+r, d*1024 + qt*256 + ch]
                    hrep = hpool.tile([128, QT_CH * 8], I16, tag="h")
                    gsl = g16_all[:, d * NCHUNK + qt * QT_CH:
                                  d * NCHUNK + (qt + 1) * QT_CH]
                    gq = gsl.rearrange("(q o) ch -> q o ch", o=16)
                    for q in range(8):
                        nc.sync.dma_start(
                            hrep[0:16, q * QT_CH:(q + 1) * QT_CH], gq[q])
                    for n in (16, 32, 64):
                        nc.sync.dma_start(hrep[n:2 * n, :], hrep[0:n, :])
                    # free-axis reorder -> wrapped layout (ch*8+q)
                    idxd = ipool.tile([128, QT_CH * 8], I16, tag="idx")
                    nc.vector.tensor_copy(
                        idxd[:].rearrange("p (ch q) -> p ch q", q=8),
                        hrep[:].rearrange("p (q ch) -> p ch q", q=8))

                    tabd = pdf_gr[d * NGR_D:(d + 1) * NGR_D, :]
                    for b in range(NCALL_QT):
                        gdst = gpool.tile([128, CALL_CH, GRAN], F32, tag="g")
                        nc.gpsimd.dma_gather(
                            gdst[:], tabd,
                            idxd[:, b * 256:(b + 1) * 256],
                            CALL_IDX, CALL_IDX, GRAN,
                            single_packet=False, queue_num=kctr % 4)
                        kctr += 1
                        chb = qt * QT_CH + b * CALL_CH   # global CH base
                        mt = epool.tile([128, CALL_CH, GRAN], BF16, tag="m")
                        r3 = r_all[:].rearrange("p (ch e) -> p ch e", e=D)[
                            :, chb:chb + CALL_CH, d:d + 1]
                        nc.vector.tensor_tensor(
                            mt[:], iob3.to_broadcast([128, CALL_CH, GRAN]),
                            r3.to_broadcast([128, CALL_CH, GRAN]),
                            op=AL.is_equal)
                        nc.vector.tensor_tensor(gdst[:], gdst[:], mt[:],
                                                op=AL.mult)
                        val = epool.tile([128, CALL_CH, 1], F32, tag="v")
                        nc.vector.tensor_reduce(val[:], gdst[:],
                                                axis=mybir.AxisListType.X,
                                                op=AL.add)
                        q3 = q_all[:].rearrange("p (ch e) -> p ch e", e=D)[
                            :, chb:chb + CALL_CH, d:d + 1]
                        nc.vector.tensor_tensor(val[:], val[:], q3, op=AL.mult)
                        osl = ot[:, chb:chb + CALL_CH].rearrange(
                            "p (ch o) -> p ch o", o=1)
                        nc.vector.tensor_tensor(osl, osl, val[:], op=AL.add)

            nc.sync.dma_start(out_ext[:], ot[:])
    return nc


def prep_inputs(wi, cond, w0, b0, w1, b1, w2, b2, w3, b3, pdf, fac):
    """Host-side sharding + layout. Returns in_maps (list of 8 dicts)."""
    perm = np.concatenate([np.arange(D),                 # weight
                           D + 2 * np.arange(D),         # vx
                           D + 2 * np.arange(D) + 1,     # vy
                           3 * D + np.arange(D)])        # z
    w3r = w3[:, perm].astype(np.float32)
    b3r = b3[perm].astype(np.float32)

    def packw(w, kk, rstep):
        t = np.zeros((128, 128), np.float32)
        for g in range(NQ):
            t[rstep * g:rstep * g + kk, 32 * g:32 * g + 32] = w
        return t

    w0p = packw(w0, CIN, CIN)
    w1p = packw(w1, HID, HID)
    w2p = packw(w2, HID, HID)
    w3p = packw(w3r, HID, HID)
    b012 = np.zeros((128, 3), np.float32)
    b3p = np.zeros((128, 1), np.float32)
    for g in range(NQ):
        for li, b in enumerate((b0, b1, b2)):
            b012[32 * g:32 * g + 32, li] = b
        b3p[32 * g:32 * g + 32, 0] = b3r
    ident = np.eye(128, dtype=np.float32)
    iota64 = np.broadcast_to(
        np.arange(GRAN, dtype=np.float32).reshape(1, GRAN), (128, GRAN)).copy()
    fac_in = np.ascontiguousarray(
        fac.astype(np.float32).reshape(1, ANG * D))   # col z*D + d

    # granule table: row d*32768 + z*4096 + iy*8 + xblk
    pdf_gr = np.ascontiguousarray(
        pdf.astype(np.float32).transpose(1, 0, 2, 3).reshape(D * NGR_D, GRAN))

    in_maps = []
    for c in range(NC_N):
        sl = slice(c * BC, (c + 1) * BC)
        cond_c = cond[sl].reshape(NQ, QS, CIN)
        cond_t = np.ascontiguousarray(
            cond_c.transpose(0, 2, 1).reshape(NQ * CIN, QS))
        wi_c = wi[sl]
        g_, s_, cc_, p_ = np.meshgrid(np.arange(NQ), np.arange(NT), np.arange(4),
                                      np.arange(128), indexing='ij')
        samp = (g_ * QS + s_ * 512 + cc_ * 128 + p_)
        chunk = (s_ * 16 + cc_ * 4 + g_)
        wx = np.zeros((128, NCHUNK, D), np.float32)
        wy = np.zeros((128, NCHUNK, D), np.float32)
        wx[p_.ravel(), chunk.ravel()] = wi_c[samp.ravel(), 0:1]
        wy[p_.ravel(), chunk.ravel()] = wi_c[samp.ravel(), 1:2]
        wx = wx.reshape(128, NT, 128)
        wy = wy.reshape(128, NT, 128)
        wi_xy = np.concatenate([wx, wy], axis=2).reshape(128, NT * 256)
        in_maps.append(dict(
            cond_t=cond_t, wi_xy=np.ascontiguousarray(wi_xy),
            w0p=w0p, w1p=w1p, w2p=w2p, w3p=w3p, b012=b012, b3p=b3p,
            ident=ident, iota64=iota64, fac_in=fac_in, pdf_gr=pdf_gr))
    return in_maps


def unshard_output(results):
    out = np.empty(B, np.float32)
    g_, s_, cc_, p_ = np.meshgrid(np.arange(NQ), np.arange(NT), np.arange(4),
                                  np.arange(128), indexing='ij')
    samp = (g_ * QS + s_ * 512 + cc_ * 128 + p_).ravel()
    chunk = (s_ * 16 + cc_ * 4 + g_).ravel()
    for c in range(NC_N):
        o = results[c]["out"]  # [128, NCHUNK]
        out[c * BC + samp] = o[p_.ravel(), chunk]
    return out


_CACHE = {}


def kernel(**inputs):
    if 'nc' not in _CACHE:
        _CACHE['nc'] = build_kernel()
    nc = _CACHE['nc']
    if not nc.is_finalized():
        nc.finalize()
    in_maps = prep_inputs(**{k: np.asarray(v) for k, v in inputs.items()})
    r = run_bass_kernel_spmd(nc, in_maps, list(range(NC_N)),
                             trace=bool(os.environ.get("KTRACE")))
    if r.exec_time_ns:
        print(f"HW exec time: {r.exec_time_ns} ns")
    if os.environ.get("KTRACE") and r.instructions_and_trace:
        try:
            import pickle
            insts, tracep = r.instructions_and_trace
            rows = [(i.name, str(i.engine), i.timestamp, i.duration,
                     i.op_name() if callable(getattr(i, 'op_name', None))
                     else getattr(i, 'op_name', ''))
                    for i in insts]
            with open('/tmp/ktrace_insts.pkl', 'wb') as f:
                pickle.dump(rows, f)
            print(f"trace: {tracep} profile_json: {r.profile_json} "
                  f"n_insts: {len(rows)}")
        except Exception as e:
            print("trace dump failed:", e)
    return unshard_output(r.results)


if __name__ == "__main__":
    pass
